# revision 56
# baseline (speedup 1.0000x reference)
"""Trainium2 Bass kernel for nn_GroupAttentionLayer (sparse block attention).

Strategy (8 NeuronCores, SPMD): core i handles batch b=i//2, query-pixel
half h=i%2 (2048 query pixels each). All heavy tensors are bf16 (PSUM
accumulation stays fp32), attention tiles are 1024 queries wide, and the
epilogue is fully SHARDED (the 1x1 conv + BN/softmax reductions are
pointwise, so no core ever needs the full tensor):

    scores^T[k,q] = Qc[:,k].T @ Xq[:,q]      (PE, contract channels, bf16)
    E = exp(scores/8)                        (ACT, bf16 out)
    D_bcast = blockmap.T @ E                 (PE, per-64-block sums, into a
                                              2-bank PSUM pair tile)
    R = 1/D for an iteration PAIR            (one 1024-wide DVE reciprocal)
    A = E * R                                (halves split DVE / Pool)
    agg^T[c,q] += x_blk[k,:].T @ A           (PE, PSUM accum, Conv_K folded)

The PE instruction stream is software-pipelined (scores issued 8 iters
ahead of the dependent weighted-sum accumulate; exp halves live in
separate tiles so the DVE/Pool normalize multiplies are provably
independent) so the in-order engine queues never stall on cross-engine
dependencies.  Dependency-free
"warmer" matmuls keep the PE clock-gate at full speed through the
collective windows.

Four tiny AllGathers (cheaper than AllReduce under the collective cost
model) carry the cross-core reductions: BN_Q batch stats, BN_1 batch
stats, per-batch spatial-softmax exp sums, BN_O batch stats. BN shifts
that feed the spatial softmax cancel algebraically (softmax is
shift-invariant) and conv biases cancel inside training-mode BN, so
neither is computed. A per-core one-hot mask input selects this core's
batch pair out of the gathered exp sums.

Host side: shards/transposes/bf16-converts inputs with numpy, assembles
the output from the 8 per-core channel-major shards.
"""

import numpy as np

B, H, W, C = 4, 64, 64, 128
RF = 8
EPS = 1e-3
ALPHA = 0.1
N_CORES = 8
HWPIX = H * W             # 4096 pixels per batch
QSH = HWPIX * B // N_CORES  # 2048 query pixels per core
PW = W + 2                # 66, padded row width
PADN = PW * (H + 2)       # 4356 padded columns
NKT = HWPIX // 128        # 32 key tiles per batch
TW = 512                  # attention tile width (queries; matmul out is
                          # capped at 512 fp32 = one PSUM bank)
NQT = QSH // TW           # 4 query tiles per core
NCC = HWPIX // TW         # 8 conv chunks (1 block-row each)
NIT = NQT * NKT           # 128 inner iterations

_CACHE = {}


def _build_program():
    import concourse.bacc as bacc
    import concourse.tile as tile
    from concourse import mybir

    f32 = mybir.dt.float32
    bf16 = mybir.dt.bfloat16
    AF = mybir.ActivationFunctionType
    OP = mybir.AluOpType
    AX = mybir.AxisListType

    nc = bacc.Bacc("TRN2", target_bir_lowering=False, debug=False,
                   enable_asserts=True, num_devices=N_CORES)

    # per-core inputs (bf16 heavy data, fp32 vectors)
    d_xnat = nc.dram_tensor("xnat", [HWPIX, C], bf16, kind="ExternalInput").ap()
    d_xqT = nc.dram_tensor("xqT", [C, QSH], bf16, kind="ExternalInput").ap()
    d_xpadT = nc.dram_tensor("xpadT", [C, PADN], bf16, kind="ExternalInput").ap()
    d_wq9 = nc.dram_tensor("wq9", [9, C, C], bf16, kind="ExternalInput").ap()
    d_wk = nc.dram_tensor("wk", [C, C], bf16, kind="ExternalInput").ap()
    d_wo = nc.dram_tensor("wo", [C, C], bf16, kind="ExternalInput").ap()
    d_vecs = nc.dram_tensor("vecs", [6, C], f32, kind="ExternalInput").ap()
    d_msk = nc.dram_tensor("msk", [8, C], f32, kind="ExternalInput").ap()
    d_bm = nc.dram_tensor("bm", [C, C], bf16, kind="ExternalInput").ap()
    # output: this core's channel-major shard
    d_outT = nc.dram_tensor("outT", [C, QSH], bf16,
                            kind="ExternalOutput").ap()

    with tile.TileContext(nc) as tc:
        with tc.tile_pool(name="const", bufs=1) as const, \
             tc.tile_pool(name="big", bufs=1) as big, \
             tc.tile_pool(name="work", bufs=7) as work, \
             tc.tile_pool(name="tmp2", bufs=3) as tmp2p, \
             tc.tile_pool(name="small", bufs=2) as small, \
             tc.tile_pool(name="ps", bufs=2, space="PSUM") as ps, \
             tc.tile_pool(name="psD", bufs=2, space="PSUM") as psD2, \
             tc.tile_pool(name="psA", bufs=2, space="PSUM") as psA, \
             tc.tile_pool(name="dram", bufs=1, space="DRAM") as dram:

            # ---------------- loads ----------------
            # conv-critical loads (Wq + Xpad row bands) alone and first on
            # the SP/ACT queues; everything else on the DVE/Pool queues so
            # the tile framework's counting-sem waits for the conv matmuls
            # cover only the conv inputs (and the one serialized DMA device
            # serves them first)
            Wq_s = const.tile([C, 9, C], bf16)
            nc.scalar.dma_start(
                Wq_s[:], d_wq9.rearrange("t ci co -> ci t co"))
            # four SEPARATE overlapping row-band tiles (rows [16r, 16r+18))
            # so conv chunk c only waits for band c//2's DMA, not all four
            d_xpr = d_xpadT.rearrange("p (r c) -> p r c", r=H + 2)
            Xbands = []
            for rr in range(4):
                xb_t = big.tile([C, 18 * PW], bf16, tag=f"xband{rr}")
                nc.sync.dma_start(
                    xb_t[:].rearrange("p (r c) -> p r c", r=18),
                    d_xpr[:, 16 * rr:16 * rr + 18, :])
                Xbands.append(xb_t)
            Xq = big.tile([C, QSH], bf16)
            nc.gpsimd.dma_start(Xq[:], d_xqT[:])
            V = const.tile([C, 6], f32)
            nc.gpsimd.dma_start(V[:], d_vecs.rearrange("v c -> c v"))
            Msk = const.tile([C, 8], f32)
            nc.gpsimd.dma_start(Msk[:], d_msk.rearrange("v c -> c v"))
            Bb = const.tile([C, C], bf16)
            nc.gpsimd.dma_start(Bb[:], d_bm[:])
            Wk_s = const.tile([C, C], bf16)
            nc.gpsimd.dma_start(Wk_s[:], d_wk[:])
            Wo_s = const.tile([C, C], bf16)
            nc.gpsimd.dma_start(Wo_s[:], d_wo[:])
            Xnat = big.tile([128, NKT, C], bf16)
            nc.gpsimd.dma_start(
                Xnat[:], d_xnat.rearrange("(t p) c -> p t c", p=128))
            eps_t = const.tile([C, 1], f32)
            nc.vector.memset(eps_t[:], EPS)
            # load the ln+exp activation table once, up front: every ACT
            # function used below (Exp, Ln, Identity) lives in this set, so
            # the compiler pass inserts no further mid-chain table reloads
            from concourse.hw_specs import get_activation_tables
            _tabs = list(get_activation_tables(nc.m.arch))
            _set_id = _tabs.index("natural_log_exp_and_others")
            nc.scalar.add_instruction(mybir.InstLoadActFuncSet(
                name=nc.get_next_instruction_name(), ins=[], outs=[],
                act_func_set_id=_set_id))
            Wz = const.tile([C, TW], bf16)
            nc.vector.memset(Wz[:], 0.0)

            def warm(n):
                # dependency-free matmuls that keep the PE pstate at full
                # clock through windows where real work is blocked
                for _ in range(n):
                    wp = ps.tile([C, TW], f32, tag="ps")
                    nc.tensor.matmul(wp[:], Wz[:, :C], Wz[:],
                                     start=True, stop=True)

            warm(8)


            # ---------------- CBL_Q: conv3x3 + batch stats ----------------
            # Conv output in BLOCK-MAJOR key order: chunk t covers block rows
            # n=2t,2t+1; column n*512 + m*64 + p*8 + q is pixel (8n+p, 8m+q).
            Zq = big.tile([C, NCC, TW], bf16)
            qstats = small.tile([C, NCC, 6], f32)
            for t in range(NCC):
                pq = ps.tile([C, TW], f32, tag="ps")
                Xbv = Xbands[t // 2][:].rearrange("p (r c) -> p r c", r=18)
                rb = 8 * (t % 2)  # chunk row base within its band
                for tap in range(9):
                    dh, dw = tap // 3 - 1, tap % 3 - 1
                    rhs = Xbv[:, rb + 1 + dh: rb + 9 + dh,
                              1 + dw: 65 + dw].rearrange(
                                  "c p (m q) -> c m p q", m=8)
                    nc.tensor.matmul(pq[:], Wq_s[:, tap, :], rhs,
                                     start=(tap == 0), stop=(tap == 8))
                nc.vector.bn_stats(qstats[:, t, :], pq[:])
                nc.scalar.copy(Zq[:, t, :], pq[:])

            qmv = small.tile([C, 2], f32)
            nc.vector.bn_aggr(qmv[:], qstats[:])
            # partial sums for the cross-core stats:
            #   sums[:,0] = mean * 4096 ; sums[:,1] = (var + mean^2) * 4096
            sums = small.tile([C, 2], f32)
            nc.vector.tensor_scalar_mul(sums[:, 0:1], qmv[:, 0:1], float(HWPIX))
            m2 = small.tile([C, 1], f32)
            nc.vector.tensor_mul(m2[:], qmv[:, 0:1], qmv[:, 0:1])
            nc.vector.tensor_add(m2[:], m2[:], qmv[:, 1:2])
            nc.vector.tensor_scalar_mul(sums[:, 1:2], m2[:], float(HWPIX))

            grp = [list(range(N_CORES))]

            def gather8(name, src, width, warm_n=0):
                """AllGather a [C,width] fp32 tile -> [C,8,width] on-chip."""
                t_in = dram.tile([C, width], f32, tag=f"{name}_in")
                t_out = dram.tile([N_CORES * C, width], f32,
                                  addr_space="Shared", tag=f"{name}_out")
                nc.sync.dma_start(t_in[:], src[:])
                nc.gpsimd.collective_compute(
                    "AllGather", OP.bypass, replica_groups=grp,
                    ins=[t_in.opt()], outs=[t_out.opt()])
                if warm_n:
                    warm(warm_n)
                g = small.tile([C, 8, width], f32, tag=f"{name}_g")
                nc.sync.dma_start(
                    g[:], t_out[:].rearrange("(r p) s -> p r s", r=N_CORES))
                return g

            def bn_affine(gst8, tot, gamma, beta=None):
                """gst8: [C,8,2] gathered (sum, sumsq) partials. Returns
                (a, b): a = gamma*rsqrt(var+eps), b = beta - a*mean (b=None
                when beta is None -- the shift cancels downstream)."""
                gsum = small.tile([C, 2], f32, tag="gsum")
                nc.vector.tensor_reduce(
                    gsum[:], gst8[:].rearrange("c r s -> c s r"),
                    axis=AX.X, op=OP.add)
                sc = small.tile([C, 2], f32, tag="scmom")
                nc.vector.tensor_scalar_mul(sc[:], gsum[:], 1.0 / tot)
                negvar = small.tile([C, 1], f32, tag="negvar")
                nc.vector.scalar_tensor_tensor(negvar[:], sc[:, 0:1],
                                               sc[:, 0:1], sc[:, 1:2],
                                               op0=OP.mult, op1=OP.subtract)
                # rsqrt(var+eps) = exp(-0.5*ln(var+eps)); ln/exp share one
                # ACT table set, so no LoadActFuncSet in this chain
                lnv = small.tile([C, 1], f32, tag="lnv")
                nc.scalar.activation(lnv[:], negvar[:], AF.Ln,
                                     scale=-1.0, bias=eps_t[:])
                rstd = small.tile([C, 1], f32, tag="rstd")
                nc.scalar.activation(rstd[:], lnv[:], AF.Exp, scale=-0.5)
                a = small.tile([C, 1], f32, tag="acoef")
                nc.vector.tensor_mul(a[:], rstd[:], gamma)
                if beta is None:
                    return a, None
                b = small.tile([C, 1], f32, tag="bcoef")
                nc.vector.tensor_scalar(b[:], sc[:, 0:1], a[:], -1.0,
                                        op0=OP.mult, op1=OP.mult)
                nc.vector.tensor_add(b[:], b[:], beta)
                return a, b

            gstq = gather8("stq", sums, 2, warm_n=48)
            aq, bq = bn_affine(gstq, float(HWPIX * N_CORES),
                               V[:, 0:1], V[:, 1:2])

            # q = leaky(aq*z + bq); Zq is already block-major. Chunk 0 is
            # emitted here; chunks 1-3 are interleaved into the main loop
            # (issued a few iterations before their keys are needed) so the
            # in-order ACT queue never blocks exp_0 behind them.
            Qc = big.tile([C, HWPIX], bf16)
            Qv = Qc[:].rearrange("p (t f) -> p t f", f=TW)

            def affine_chunk(t):
                tmp = tmp2p.tile([C, TW], f32, tag="tmp2")
                nc.scalar.activation(tmp[:], Zq[:, t, :], AF.Identity,
                                     scale=aq[:], bias=bq[:])
                nc.vector.scalar_tensor_tensor(Qv[:, t, :], tmp[:],
                                               ALPHA, tmp[:],
                                               op0=OP.mult, op1=OP.max)

            # chunk 0 in two slices: the loop's first scores matmul only
            # reads Qc[:, :128], so a 128-wide head slice unblocks it early
            tmp0 = tmp2p.tile([C, TW], f32, tag="tmp2")
            nc.scalar.activation(tmp0[:, :128], Zq[:, 0, :128], AF.Identity,
                                 scale=aq[:], bias=bq[:])
            nc.vector.scalar_tensor_tensor(Qv[:, 0, :128], tmp0[:, :128],
                                           ALPHA, tmp0[:, :128],
                                           op0=OP.mult, op1=OP.max)
            nc.scalar.activation(tmp0[:, 128:], Zq[:, 0, 128:], AF.Identity,
                                 scale=aq[:], bias=bq[:])
            nc.vector.scalar_tensor_tensor(Qv[:, 0, 128:], tmp0[:, 128:],
                                           ALPHA, tmp0[:, 128:],
                                           op0=OP.mult, op1=OP.max)

            # ---------------- attention main loop (software-pipelined) ----
            z1 = big.tile([C, NQT, TW], bf16)
            qs1z = small.tile([C, NQT, 6], f32)
            paggs = [None] * NQT
            E0s = [None] * (NIT // 2)   # [C,TW] bf16, even-iter exp/attn
            E1s = [None] * (NIT // 2)   # [C,TW] bf16, odd-iter exp/attn
            psDs = [None] * (NIT // 2)  # [C,2,TW] f32 psum pair tiles
            with nc.allow_low_precision(reason="attn weights in bf16"):
                for i in range(NIT + 10):
                    qt, kt = i // NKT, i % NKT
                    if qt == 0 and kt % 4 == 2 and kt < 28:
                        affine_chunk(kt // 4 + 1)
                    # stage 0: scores (+ per-qt Conv_K group start)
                    if i < NIT:
                        if kt == 0:
                            pagg_n = psA.tile([C, TW], f32, tag="agg")
                            paggs[qt] = pagg_n
                            nc.tensor.matmul(
                                pagg_n[:], Wk_s[:],
                                Xq[:, qt * TW:(qt + 1) * TW],
                                start=True, stop=False)
                        psS = ps.tile([C, TW], f32, tag="ps")
                        nc.tensor.matmul(psS[:],
                                         Qc[:, kt * 128:(kt + 1) * 128],
                                         Xq[:, qt * TW:(qt + 1) * TW],
                                         start=True, stop=True)
                        if i % 2 == 0:
                            Epn = work.tile([C, TW], bf16, tag="E0")
                            E0s[i // 2] = Epn
                            nc.scalar.activation(Epn[:], psS[:], AF.Exp,
                                                 scale=1.0 / RF)
                        else:
                            Epn = work.tile([C, TW], bf16, tag="E1")
                            E1s[i // 2] = Epn
                            nc.scalar.activation(Epn[:], psS[:], AF.Exp,
                                                 scale=1.0 / RF)
                    # stage 1: block-sum matmul; after a pair of D's, one
                    # 1024-wide reciprocal + multiply. (TensorTensor divide
                    # is not a legal DVE op; TensorScalar is not legal on
                    # Pool; Pool multiplies are SBUF-only -> all legal here.)
                    if 1 <= i < NIT + 1:
                        j = i - 1
                        p, h = j // 2, j % 2
                        if h == 0:
                            psDn = psD2.tile([C, 2, TW], f32, tag="d")
                            psDs[p] = psDn
                        Eh = E0s[p] if h == 0 else E1s[p]
                        nc.tensor.matmul(psDs[p][:, h, :], Bb[:], Eh[:],
                                         start=True, stop=True)
                        if h == 1:
                            R = work.tile([C, 2, TW], bf16, tag="R")
                            nc.vector.reciprocal(
                                R[:].rearrange("c a b -> c (a b)"),
                                psDs[p][:].rearrange("c a b -> c (a b)"))
                            psDs[p] = None
                            # halves in TRUE parallel (separate tiles, so no
                            # tile-granular false dependency): DVE takes
                            # half0 (needed first by the agg matmul), Pool
                            # takes half1
                            eng0 = nc.gpsimd if p % 4 == 3 else nc.vector
                            eng0.tensor_mul(E0s[p][:], E0s[p][:],
                                            R[:, 0, :])
                            nc.gpsimd.tensor_mul(E1s[p][:], E1s[p][:],
                                                 R[:, 1, :])
                    # stage 2: weighted-sum accumulate (10 steps behind so
                    # the pair-normalize has drained)
                    if i >= 10:
                        j = i - 10
                        jqt, jkt = j // NKT, j % NKT
                        Ejh = E0s[j // 2] if j % 2 == 0 else E1s[j // 2]
                        nc.tensor.matmul(paggs[jqt][:], Xnat[:, jkt, :],
                                         Ejh[:], start=False,
                                         stop=(jkt == NKT - 1))
                        if j % 2 == 0:
                            E0s[j // 2] = None
                        else:
                            E1s[j // 2] = None
                        if jkt == NKT - 1:
                            nc.vector.bn_stats(qs1z[:, jqt, :],
                                               paggs[jqt][:])
                            nc.scalar.copy(z1[:, jqt, :], paggs[jqt][:])
                            paggs[jqt] = None

            # ---------------- epilogue (sharded) ----------------
            # partial BN_1 sums for this core's 2048 pixels
            mv1 = small.tile([C, 2], f32)
            nc.vector.bn_aggr(mv1[:], qs1z[:])
            sums1 = small.tile([C, 2], f32)
            nc.vector.tensor_scalar_mul(sums1[:, 0:1], mv1[:, 0:1], float(QSH))
            m21 = small.tile([C, 1], f32)
            nc.vector.tensor_mul(m21[:], mv1[:, 0:1], mv1[:, 0:1])
            nc.vector.tensor_add(m21[:], m21[:], mv1[:, 1:2])
            nc.vector.tensor_scalar_mul(sums1[:, 1:2], m21[:], float(QSH))

            gst1 = gather8("st1", sums1, 2)
            # only a1 = g1*rsqrt(var+eps) matters: the spatial softmax is
            # invariant to the BN_1 shift (and to bt1)
            a1, _b1 = bn_affine(gst1, float(B * HWPIX), V[:, 2:3])

            # E1 = exp(a1 * z1) in one ACT pass; accum gives the shard sum
            E1 = big.tile([C, NQT, TW], bf16)
            esum = small.tile([C, 1], f32)
            nc.scalar.activation(E1[:].rearrange("p a b -> p (a b)"),
                                 z1[:].rearrange("p a b -> p (a b)"),
                                 AF.Exp, scale=a1[:], accum_out=esum[:])

            ges = gather8("es", esum, 1)
            # this core's spatial-softmax denominator: the two partials of
            # its own batch pair, selected by the per-core one-hot mask
            gsel = small.tile([C, 8], f32)
            nc.vector.tensor_mul(gsel[:], ges[:, :, 0], Msk[:])
            den = small.tile([C, 1], f32)
            nc.vector.tensor_reduce(den[:], gsel[:], axis=AX.X, op=OP.add)
            rb = small.tile([C, 1], f32)
            nc.vector.reciprocal(rb[:], den[:])

            # y = E1 * rb (in place), then CBL_O conv + batch stats
            # the softmax normalization is per INPUT channel, so it folds
            # into the conv weights: zO = (Wo * rb)^T @ E1 -- one tiny
            # per-partition scale instead of four full-width normalizes
            Wo2 = small.tile([C, C], bf16)
            with nc.allow_low_precision(reason="softmax weights bf16"):
                nc.vector.tensor_scalar_mul(Wo2[:], Wo_s[:], rb[:])
            zO = big.tile([C, NQT, TW], bf16)
            stO = small.tile([C, NQT, 6], f32)
            for t in range(NQT):
                # alternate PSUM pools: 4 independent output slots so the
                # four conv matmuls run back-to-back instead of waiting for
                # each other's bn_stats/copy drain
                if t % 2 == 0:
                    pzo = ps.tile([C, TW], f32, tag="ps")
                    pz = pzo[:]
                else:
                    pzoD = psD2.tile([C, 2, TW], f32, tag="d")
                    pz = pzoD[:, 0, :]
                nc.tensor.matmul(pz, Wo2[:], E1[:, t, :],
                                 start=True, stop=True)
                nc.vector.bn_stats(stO[:, t, :], pz)
                nc.scalar.copy(zO[:, t, :], pz)

            mvO = small.tile([C, 2], f32)
            nc.vector.bn_aggr(mvO[:], stO[:])
            sumsO = small.tile([C, 2], f32)
            nc.vector.tensor_scalar_mul(sumsO[:, 0:1], mvO[:, 0:1], float(QSH))
            m2O = small.tile([C, 1], f32)
            nc.vector.tensor_mul(m2O[:], mvO[:, 0:1], mvO[:, 0:1])
            nc.vector.tensor_add(m2O[:], m2O[:], mvO[:, 1:2])
            nc.vector.tensor_scalar_mul(sumsO[:, 1:2], m2O[:], float(QSH))

            gstO = gather8("stO", sumsO, 2)
            aO, bO = bn_affine(gstO, float(B * HWPIX), V[:, 4:5], V[:, 5:6])

            OUT = big.tile([C, NQT, TW], bf16)
            for t in range(NQT):
                tmp = tmp2p.tile([C, TW], f32, tag="tmp2")
                nc.scalar.activation(tmp[:], zO[:, t, :], AF.Identity,
                                     scale=aO[:], bias=bO[:])
                nc.vector.scalar_tensor_tensor(OUT[:, t, :], tmp[:], ALPHA,
                                               tmp[:], op0=OP.mult,
                                               op1=OP.max)
                eng = nc.sync if t % 2 == 0 else nc.scalar
                eng.dma_start(d_outT[:, t * TW:(t + 1) * TW], OUT[:, t, :])

    nc.compile()
    return nc


def _get_runner():
    if "runner" in _CACHE:
        return _CACHE["runner"]
    import jax
    import numpy as np
    from jax.sharding import Mesh, PartitionSpec
    from jax.experimental.shard_map import shard_map
    from concourse import mybir
    from concourse.bass2jax import (_bass_exec_p, install_neuronx_cc_hook,
                                    partition_id_tensor)

    nc = _build_program()
    install_neuronx_cc_hook()

    in_names, out_names, out_avals, zero_outs = [], [], [], []
    partition_name = nc.partition_id_tensor.name if nc.partition_id_tensor else None
    for alloc in nc.m.functions[0].allocations:
        if not isinstance(alloc, mybir.MemoryLocationSet):
            continue
        name = alloc.memorylocations[0].name
        if alloc.kind == "ExternalInput":
            if name != partition_name:
                in_names.append(name)
        elif alloc.kind == "ExternalOutput":
            shape = tuple(alloc.tensor_shape)
            dtype = mybir.dt.np(alloc.dtype)
            out_names.append(name)
            out_avals.append(jax.core.ShapedArray(shape, dtype))
            zero_outs.append(np.zeros(shape, dtype))
    n_params = len(in_names)
    n_outs = len(out_avals)
    all_in_names = list(in_names) + list(out_names)
    if partition_name is not None:
        all_in_names.append(partition_name)

    def _body(*args):
        operands = list(args)
        if partition_name is not None:
            operands.append(partition_id_tensor())
        outs = _bass_exec_p.bind(
            *operands,
            out_avals=tuple(out_avals),
            in_names=tuple(all_in_names),
            out_names=tuple(out_names),
            lowering_input_output_aliases=(),
            sim_require_finite=True,
            sim_require_nnan=True,
            nc=nc,
        )
        return tuple(outs)

    donate = tuple(range(n_params, n_params + n_outs))
    try:
        devices = jax.devices("axon")[:N_CORES]
    except RuntimeError:
        devices = jax.devices()[:N_CORES]
    mesh = Mesh(np.asarray(devices), ("core",))
    in_specs = (PartitionSpec("core"),) * (n_params + n_outs)
    out_specs = (PartitionSpec("core"),) * n_outs
    sharded = jax.jit(
        shard_map(_body, mesh=mesh, in_specs=in_specs, out_specs=out_specs,
                  check_rep=False),
        donate_argnums=donate, keep_unused=True)

    def run(in_maps):
        per_core = [[np.asarray(m[name]) for name in in_names] for m in in_maps]
        concat_in = [np.concatenate([per_core[c][i] for c in range(N_CORES)],
                                    axis=0) for i in range(n_params)]
        concat_zeros = [np.zeros((N_CORES * z.shape[0], *z.shape[1:]), z.dtype)
                        for z in zero_outs]
        out_arrs = jax.block_until_ready(sharded(*concat_in, *concat_zeros))
        return [
            {name: np.asarray(out_arrs[i]).reshape(N_CORES, *out_avals[i].shape)[c]
             for i, name in enumerate(out_names)}
            for c in range(N_CORES)
        ]

    _CACHE["runner"] = run
    return run


def _make_blockmap():
    bm = np.zeros((C, C), np.float32)
    idx = np.arange(C)
    bm[(idx[:, None] // 64) == (idx[None, :] // 64)] = 1.0
    return bm


def kernel(x, Wq, bq, gq, btq, Wk, bk, g1, bt1, Wo, bo, go, bto):
    """Full inputs -> full output. Conv biases cancel inside training-mode
    BN (the mean subtraction removes any per-channel constant), so bq/bk/bo
    never enter the device program."""
    import ml_dtypes
    bf16 = ml_dtypes.bfloat16

    x = np.asarray(x, np.float32)
    run = _get_runner()

    wq9 = np.ascontiguousarray(
        np.asarray(Wq, np.float32).reshape(9, C, C)).astype(bf16)
    wk = np.ascontiguousarray(
        np.asarray(Wk, np.float32).reshape(C, C)).astype(bf16)
    wo = np.ascontiguousarray(
        np.asarray(Wo, np.float32).reshape(C, C)).astype(bf16)
    vecs = np.ascontiguousarray(np.stack([
        np.asarray(v, np.float32) for v in (gq, btq, g1, bt1, go, bto)]))
    bm = _make_blockmap().astype(bf16)

    # block-major key permutation: index (n,m,p,q) -> pixel (8n+p, 8m+q)
    perm = np.arange(HWPIX).reshape(8, 8, 8, 8).transpose(0, 2, 1, 3).reshape(-1)

    in_maps = []
    for core in range(N_CORES):
        b, h = core // 2, core % 2
        xb = np.ascontiguousarray(x[b].reshape(HWPIX, C))
        xbT = xb.T  # [C, HWPIX]
        xqT = np.ascontiguousarray(xbT[:, h * QSH:(h + 1) * QSH]).astype(bf16)
        xpadT = np.zeros((C, H + 2, W + 2), np.float32)
        xpadT[:, 1:H + 1, 1:W + 1] = xbT.reshape(C, H, W)
        msk = np.zeros((8, C), np.float32)
        msk[2 * b] = 1.0
        msk[2 * b + 1] = 1.0
        in_maps.append({
            "xnat": np.ascontiguousarray(xb[perm]).astype(bf16),
            "xqT": xqT,
            "xpadT": np.ascontiguousarray(
                xpadT.reshape(C, PADN)).astype(bf16),
            "wq9": wq9, "wk": wk, "wo": wo, "vecs": vecs, "msk": msk,
            "bm": bm,
        })

    res = run(in_maps)
    full = np.empty((B, HWPIX, C), np.float32)
    for core in range(N_CORES):
        b, h = core // 2, core % 2
        full[b, h * QSH:(h + 1) * QSH, :] = \
            res[core]["outT"].T.astype(np.float32)
    return full.reshape(B, H, W, C)
